# revision 63
# baseline (speedup 1.0000x reference)
"""MoE LoRA adapter layer (top-2 routed, E=8 experts, R=16) on 8 TRN2 NeuronCores.

Strategy: data-parallel over batch B=32 -> 4 batches/core; router + LoRA
weights replicated (tiny). E*R = 128 = partition width, so the per-expert
LoRA down/up projections stack into two dense matmuls:
    P1T[er, t] = D_all[er, :] @ x[t, :]^T          (contract H=1024)
    wT[h, t]   = U_all[er, h]^T @ (gate * P1T)     (contract ER=128)
The expert sum IS the matmul contraction; gates (exactly 0 off the top-2)
are folded in by scaling P1T columns per batch (ACT copy with per-partition
scale).

Everything runs in the transposed domain so the PE never transposes:
x is shipped pre-transposed from the host as xt[p, c, k, t] (bf16, c = batch
quarter) and y is stored transposed as y_out[k, p, t], un-transposed on the
host. The residual add yT = wT + xT (DVE) reuses the same xt tiles MM1
consumed, so HBM traffic stays at the ~8.4 MiB minimum per core.

Pipeline granularity is one batch (512 tokens), MM1+gate-scale emitted one
batch ahead of MM2. PSUM evacuation of the 4 k-pair tiles per batch splits
across engines (pair order DVE-direct / ACT-copy+DVE-2x-add x2 /
DVE-direct) so no single engine paces the pipeline. x loads + stores ride
the sync HWDGE ring; the packed gates input and d/u weights ride the
scalar ring concurrently. At ~40us the kernel sits at the system floor:
~7us NEFF boot + ~27us of HBM streaming (9.1 MB at ~340 GB/s) + ~3.5us
end-of-kernel barrier.
"""

import sys

if "/opt/trn_rl_repo" not in sys.path:
    sys.path.insert(0, "/opt/trn_rl_repo")

import numpy as np
import ml_dtypes

import concourse.tile as tile
from concourse import bacc, mybir
from concourse.bass_utils import run_bass_kernel_spmd

B, L, H = 32, 512, 1024
E, R, TOP_K = 8, 16, 2
N_CORES = 8
NB = B // N_CORES          # batches (quarters) per core = 4
T = NB * L                 # tokens per core = 2048
P = 128                    # partitions
NK = H // P                # H k-tiles = 8

F32 = mybir.dt.float32
BF16 = mybir.dt.bfloat16
BF16_NP = ml_dtypes.bfloat16

# packed gates-input layout: [clsT | rwt | idn4 | rep] along free dim (f32)
G_CLS = 0
G_RWT = G_CLS + NK * NB        # 32
G_IDN = G_RWT + NK * E         # 96  (4x4 identity corner is all gates use)
G_REP = G_IDN + NB             # 100
G_END = G_REP + P              # 228

_COMPILED = None


def _build():
    """Build + compile the single-core program (same on all 8 cores)."""
    nc = bacc.Bacc("TRN2", target_bir_lowering=False, debug=False)

    xt_in = nc.dram_tensor("xt_in", [P, NB * NK * L], BF16, kind="ExternalInput")
    gpk_in = nc.dram_tensor("gpk_in", [P, G_END], F32, kind="ExternalInput")
    d_t = nc.dram_tensor("d_t", [P, NK * P], BF16, kind="ExternalInput")
    u_in = nc.dram_tensor("u_in", [P, H], BF16, kind="ExternalInput")
    y_out = nc.dram_tensor("y_out", [NB, P * NK * L], BF16, kind="ExternalOutput")

    # y_out[c, (p k t)] -> [c, p, k, t]  (4 KB contiguous per partition/store)
    y_view = y_out.ap().rearrange("c (p k t) -> c p k t", p=P, k=NK, t=L)

    with tile.TileContext(nc) as tc:
        with (
            tc.tile_pool(name="wpool", bufs=1) as wpool,
            tc.tile_pool(name="xpool", bufs=1) as xpool,
            tc.tile_pool(name="ypool", bufs=1) as ypool,
            tc.tile_pool(name="p2pool", bufs=3) as p2pool,
            tc.tile_pool(name="wcpool", bufs=2) as wcpool,
            tc.tile_pool(name="gpool", bufs=1) as gpool,
            tc.tile_pool(name="p1_ps", bufs=2, space="PSUM") as p1_ps,
            tc.tile_pool(name="w_ps", bufs=3, space="PSUM") as w_ps,
        ):
            # ---- gates pack leads the sync ring (it gates MM2(c0), which
            # would FIFO-block interleaved MM1 work); x + stores follow on
            # sync. d/u weights ride the scalar ring concurrently. ----
            gpk = wpool.tile([P, G_END], F32, tag="gpk")
            nc.sync.dma_start(gpk[:], gpk_in.ap())
            clsT = gpk[:, G_CLS:G_RWT]
            rwt_sb = gpk[:, G_RWT:G_IDN]
            id4_sb = gpk[0:NB, G_IDN:G_REP]
            rep_sb = gpk[0:E, G_REP:G_END]

            d_sb = wpool.tile([P, NK * P], BF16, tag="d")
            nc.scalar.dma_start(d_sb[:], d_t.ap())
            u_sb = wpool.tile([P, H], BF16, tag="u")
            nc.scalar.dma_start(u_sb[:], u_in.ap())

            xt = xpool.tile([P, NB, NK, L], BF16, tag="xt")
            x_kview = xt_in.ap().rearrange(
                "p (c k t) -> c p k t", c=NB, k=NK, t=L
            )
            for c in range(NB):
                nc.sync.dma_start(xt[:, c, 0:4, :], x_kview[c][:, 0:4, :])
                nc.sync.dma_start(xt[:, c, 4:8, :], x_kview[c][:, 4:8, :])

            yt = ypool.tile([P, NK, T], BF16, tag="yt")

            # ---- gates prologue (fp32, exact top-2; clsT pre-transposed) ----
            lg_ps = w_ps.tile([P, 512], F32, tag="w")
            for k in range(NK):
                nc.tensor.matmul(
                    lg_ps[0:NB, 0:E],
                    clsT[:, k * NB : (k + 1) * NB],
                    rwt_sb[:, k * E : (k + 1) * E],
                    start=(k == 0),
                    stop=(k == NK - 1),
                )
            lg = gpool.tile([NB, E], F32, tag="lg")
            nc.vector.tensor_copy(lg[:], lg_ps[0:NB, 0:E])

            # top-2 softmax per row (E=8 along free dim)
            m1 = gpool.tile([NB, 1], F32, tag="m1")
            nc.vector.reduce_max(m1[:], lg[:], axis=mybir.AxisListType.X)
            t_sb = gpool.tile([NB, E], F32, tag="t")
            nc.vector.tensor_scalar(
                t_sb[:], lg[:], m1[:], None, op0=mybir.AluOpType.subtract
            )
            # pen = (t >= 0) * 1e30  (knocks out the argmax)
            pen = gpool.tile([NB, E], F32, tag="pen")
            nc.vector.tensor_scalar(
                pen[:], t_sb[:], 0.0, 1e30,
                op0=mybir.AluOpType.is_ge, op1=mybir.AluOpType.mult,
            )
            t2 = gpool.tile([NB, E], F32, tag="t2")
            nc.vector.tensor_sub(t2[:], t_sb[:], pen[:])
            m2 = gpool.tile([NB, 1], F32, tag="m2")
            nc.vector.reduce_max(m2[:], t2[:], axis=mybir.AxisListType.X)
            keep = gpool.tile([NB, E], F32, tag="keep")
            nc.vector.tensor_scalar(
                keep[:], t_sb[:], m2[:], None, op0=mybir.AluOpType.is_ge
            )
            ex = gpool.tile([NB, E], F32, tag="ex")
            nc.scalar.activation(ex[:], t_sb[:], mybir.ActivationFunctionType.Exp)
            eg = gpool.tile([NB, E], F32, tag="eg")
            nc.vector.tensor_mul(eg[:], ex[:], keep[:])
            s_sb = gpool.tile([NB, 1], F32, tag="s")
            nc.vector.reduce_sum(s_sb[:], eg[:], axis=mybir.AxisListType.X)
            rs = gpool.tile([NB, 1], F32, tag="rs")
            nc.vector.reciprocal(rs[:], s_sb[:])
            gts = gpool.tile([NB, E], F32, tag="gts")
            nc.vector.tensor_scalar(
                gts[:], eg[:], rs[:], None, op0=mybir.AluOpType.mult
            )

            # gatesT then replicate x16 along partitions -> gvec [128, NB]
            gt_ps = w_ps.tile([P, 512], F32, tag="w")
            nc.tensor.transpose(gt_ps[0:E, 0:NB], gts[:], id4_sb)
            gtT = gpool.tile([E, NB], F32, tag="gtT")
            nc.vector.tensor_copy(gtT[:], gt_ps[0:E, 0:NB])
            gv_ps = w_ps.tile([P, 512], F32, tag="w")
            nc.tensor.matmul(gv_ps[:, 0:NB], rep_sb[:], gtT[:])
            gvec = gpool.tile([P, NB], F32, tag="gvec")
            nc.vector.tensor_copy(gvec[:], gv_ps[:, 0:NB])

            # ---- main: per quarter (1 batch = 512 tokens); MM1+scale run one
            # quarter ahead of MM2, interleaved at k-pair granularity ----
            p1s = {}
            p2ts = {}

            def scale(c):
                p2t = p2pool.tile([P, L], BF16, tag="p2t")
                nc.scalar.activation(
                    p2t[:], p1s[c][:],
                    mybir.ActivationFunctionType.Copy,
                    scale=gvec[:, c : c + 1],
                )
                p2ts[c] = p2t

            def mm1_pair(c, kp):
                if kp == 0:
                    p1 = p1_ps.tile([P, L], F32, tag="p1")
                    p1s[c] = p1
                p1 = p1s[c]
                for k in (2 * kp, 2 * kp + 1):
                    nc.tensor.matmul(
                        p1[:],
                        d_sb[:, k * P : (k + 1) * P],
                        xt[:, c, k, :],
                        start=(k == 0),
                        stop=(k == NK - 1),
                    )

            def mm2_pair(c, kp):
                p2t = p2ts[c]
                k0 = 2 * kp
                wps = w_ps.tile([P, 2, L], F32, tag="w")
                for kk in range(2):
                    k = k0 + kk
                    nc.tensor.matmul(
                        wps[:, kk, :],
                        u_sb[:, k * P : (k + 1) * P],
                        p2t[:],
                    )
                ys = yt[:, k0 : k0 + 2, c * L : (c + 1) * L]
                if kp in (1, 2):
                    # ACT evacuates PSUM; DVE adds in all-SBUF 2x mode
                    wcp = wcpool.tile([P, 2, L], BF16, tag="wcp")
                    nc.scalar.activation(
                        wcp[:], wps[:], mybir.ActivationFunctionType.Copy
                    )
                    nc.vector.tensor_add(ys, wcp[:], xt[:, c, k0 : k0 + 2, :])
                else:
                    nc.vector.tensor_add(ys, wps[:], xt[:, c, k0 : k0 + 2, :])
                nc.sync.dma_start(
                    y_view[c][:, k0 : k0 + 2, :],
                    yt[:, k0 : k0 + 2, c * L : (c + 1) * L],
                )

            # pair-granular software pipeline: between MM2(c-1) pairs the PE
            # always has ready MM1(c) work, so a wps-recycle stall never
            # leaves it idle
            for kp in range(NK // 2):
                mm1_pair(0, kp)
            scale(0)
            for c in range(1, NB):
                for kp in range(NK // 2):
                    mm2_pair(c - 1, kp)
                    mm1_pair(c, kp)
                scale(c)
            for kp in range(NK // 2):
                mm2_pair(NB - 1, kp)

    nc.compile()
    return nc


def _weights_maps(router_w, lora_down, lora_up):
    # D_all[(e,r), h] stacked; lhsT tiles need [p, k, m] = D_all[m, k*128+p]
    d_all = lora_down.reshape(E * R, H)                       # [128, 1024]
    d_t = np.ascontiguousarray(
        d_all.T.reshape(NK, P, E * R).transpose(1, 0, 2).reshape(P, NK * P)
    ).astype(BF16_NP)
    # U_all[(e,r), h] = lora_up[e, h, r]
    u_np = np.ascontiguousarray(
        lora_up.transpose(0, 2, 1).reshape(E * R, H)
    ).astype(BF16_NP)
    # router_wT tiles [p, k, e] = router_w[e, k*128+p]
    rwt_np = np.ascontiguousarray(
        router_w.T.reshape(NK, P, E).transpose(1, 0, 2).reshape(P, NK * E)
    ).astype(np.float32)
    rep_np = np.zeros((P, P), np.float32)
    for e in range(E):
        rep_np[e, e * R : (e + 1) * R] = 1.0
    return {"d_t": d_t, "u_in": u_np, "rwt": rwt_np, "rep": rep_np}


def get_compiled():
    global _COMPILED
    if _COMPILED is None:
        _COMPILED = _build()
    return _COMPILED


def make_in_maps(x, router_w, lora_down, lora_up):
    x = np.asarray(x, np.float32)
    w = _weights_maps(
        np.asarray(router_w, np.float32),
        np.asarray(lora_down, np.float32),
        np.asarray(lora_up, np.float32),
    )
    in_maps = []
    for i in range(N_CORES):
        xc = x[i * NB : (i + 1) * NB].reshape(T, H)
        # xt[p, (c k t)] = x[c*L + t, k*128 + p]
        xt = np.ascontiguousarray(
            xc.reshape(NB, L, NK, P).transpose(3, 0, 2, 1).reshape(P, NB * NK * L)
        ).astype(BF16_NP)
        cls = x[i * NB : (i + 1) * NB, 0, :]
        # clsT[p, (k b)] = cls[b, k*128 + p]
        clsT = np.ascontiguousarray(
            cls.T.reshape(NK, P, NB).transpose(1, 0, 2).reshape(P, NK * NB)
        ).astype(np.float32)
        idn4 = np.zeros((P, NB), np.float32)
        idn4[:NB] = np.eye(NB, dtype=np.float32)
        gpk = np.concatenate([clsT, w["rwt"], idn4, w["rep"]], axis=1)
        in_maps.append({
            "xt_in": xt,
            "gpk_in": np.ascontiguousarray(gpk),
            "d_t": w["d_t"],
            "u_in": w["u_in"],
        })
    return in_maps


def unshard_core(y_np):
    """y_out [NB, P*NK*L] (bf16) -> [NB, L, H] f32."""
    y = np.asarray(y_np, np.float32).reshape(NB, P, NK, L)
    return y.transpose(0, 3, 2, 1).reshape(NB, L, H)


def kernel(x, router_w, lora_down, lora_up):
    nc = get_compiled()
    in_maps = make_in_maps(x, router_w, lora_down, lora_up)
    res = run_bass_kernel_spmd(nc, in_maps, core_ids=list(range(N_CORES)))
    out = np.empty((B, L, H), np.float32)
    for i in range(N_CORES):
        out[i * NB : (i + 1) * NB] = unshard_core(res.results[i]["y_out"])
    return out
